# revision 11
# baseline (speedup 1.0000x reference)
"""EdgeEncoder kernel for Trainium2 (8 NeuronCores, row-sharded).

Reference (per pair (i, j) of an N x N grid):
    out[h, i, j] = (1/n_ij) * sum_l mask[i,j,l] * sum_d feats[idx[i,j,l], d] * W[l, h, d]
with n_ij = max(#valid l, 1), idx in [-1, E-1], -1 = padding.

Strategy: host projects the edge table into T_l[e, 0:8] rows stored in HBM at a
256 B row stride (row 0 of each l-block zeroed for padding).  Each core owns 64
i-rows and fetches its 64*512*5 = 163840 rows with SWDGE dma_gather
(InstDMAGatherAnt, non-transpose): descriptors are generated on Q7 at
~0.34 ns/desc and executed across all 16 DMA engines, so the random-access
gather runs at DMA descriptor rate instead of the Q7 ap_gather SBUF-port rate
(~21 ns/idx) of the previous version.  Rows carry only 32 B of payload
(elem_size=8 < the 256 B the bass wrapper insists on, but the ucode only
requires the row *stride* to be a 256 B multiple — a raw InstDMAGatherAnt
bypasses the wrapper assert).

Gather stream position i lands at dst[i%128, i//128, :]; the host orders each
(chunk, l) stream so partition p = u*64+i holds pairs j = u*256 + c*KCC + kk.
DVE reduces the 5 l-tiles, multiplies by host-computed 1/n_ij (shipped
pre-broadcast over h in [p, h, k] layout, so the same op also produces the
output-major order), and results DMA out as 512 B-contiguous j-runs.
"""

import numpy as np

import concourse.bass as bass
import concourse.mybir as mybir
import concourse.tile as tile
from concourse import bacc
from concourse.bass_utils import run_bass_kernel_spmd
from concourse._compat import exact_div

N, L, H, D, E = 512, 5, 8, 16, 10000
NCORES = 8
RPC = N // NCORES            # 64 rows (i) per core
BLK = E + 1                  # rows per l-block of the table
ROWW = 64                    # f32 per table row in HBM (256 B stride)
ELEM = 8                     # f32 actually fetched per row (32 B)
KTOT = N * RPC // 128        # 256 k-columns (pairs per partition)
# Q7 scratch holds num_idxs int32 + 512 B of swizzles in 65472 B, so keep
# num_idxs per gather at 8192 (32 KB staged indices).
NCHUNK = 4
KC = KTOT // NCHUNK          # 128 k-columns per chunk
NIDX = 128 * KC              # idxs per gather instruction
f32, i16 = mybir.dt.float32, mybir.dt.int16

_cached = {}


def _dma_gather_raw(nc, out_ap, in_ap, idxs_ap, num_idxs, elem_size, elem_step,
                    queue_num=0, single_packet=False):
    """nc.gpsimd.dma_gather(transpose=False) minus the elem_size%256 assert.

    The 256 B requirement is only on the HBM row *stride* (stride_bytes_256
    descriptor field); per-row payload is a plain SDMA descriptor length.
    """
    g = nc.gpsimd
    assert idxs_ap.dtype == i16
    assert in_ap.ap[0][0] == elem_step and in_ap.ap[-1][1] == elem_size
    stride_bytes_256 = exact_div(elem_step * mybir.dt.size(in_ap.dtype), 256)
    _in_ap = g.lower_ap_dma(in_ap, for_custom_bir_dma=True)
    _idxs_ap = g.lower_ap(idxs_ap)
    _out_ap = g.lower_ap(out_ap)
    return g.add_instruction(
        mybir.InstDMAGatherAnt(
            name=nc.get_next_instruction_name(),
            ins=[*_in_ap, _idxs_ap, g.lower_val_access(g.to_reg(num_idxs))],
            outs=[_out_ap],
            transpose=False,
            num_idxs=num_idxs,
            elem_size=elem_size,
            stride_bytes_256=stride_bytes_256,
            gen_mode=0,
            single_packet=single_packet,
            queue_num=queue_num,
            sbuf_tokens_per_rank=0,
            sbuf_free_dim_per_rank=0,
            sbuf_free_dim_pad_per_rank=0,
            sbuf_byte_offset=0,
        )
    )


def build_nc():
    nc = bacc.Bacc(dynamic_dma_scratch_size=1 << 15, num_swdge_queues=4)

    idx_t = nc.dram_tensor("idxw", [128, NCHUNK * L * (NIDX // 16)], i16,
                           kind="ExternalInput")
    rcp_t = nc.dram_tensor("rcpt", [128, H, KTOT], f32, kind="ExternalInput")
    tbl_t = nc.dram_tensor("tblc", [L * BLK, ROWW], f32, kind="ExternalInput")
    out_t = nc.dram_tensor("out", [H, RPC, N], f32, kind="ExternalOutput")

    with tile.TileContext(nc) as tc:
        with (
            tc.tile_pool(name="io", bufs=1) as iopool,
            tc.tile_pool(name="gth", bufs=2) as gpool,
            tc.tile_pool(name="acc", bufs=2) as apool,
            tc.tile_pool(name="outb", bufs=2) as bpool,
        ):
            idx = iopool.tile([128, NCHUNK * L * (NIDX // 16)], i16)
            nc.sync.dma_start(out=idx[:, :], in_=idx_t[:, :])
            rcp = iopool.tile([128, H, KTOT], f32)
            nc.sync.dma_start(out=rcp[:, :, :], in_=rcp_t[:, :, :])

            scol = NIDX // 16
            for c in range(NCHUNK):
                gts = []
                for l in range(L):
                    gt = gpool.tile([128, KC, ELEM], f32, tag=f"g{l}")
                    s0 = (c * L + l) * scol
                    _dma_gather_raw(
                        nc,
                        out_ap=gt[:, :, :],
                        in_ap=tbl_t[l * BLK:(l + 1) * BLK, 0:ELEM],
                        idxs_ap=idx[:, s0:s0 + scol],
                        num_idxs=NIDX,
                        elem_size=ELEM,
                        elem_step=ROWW,
                        # queue 0 gathers hold the Pool engine for their full
                        # desc-gen (~64 us); queues 1-3 dispatch in ~1 us and
                        # generate async on their Q7 core pairs — use only 1-3
                        queue_num=1 + (c * L + l) % 3,
                    )
                    gts.append(gt[:, :, 0:H].rearrange("p k h -> p (k h)"))

                acc = apool.tile([128, KC * H], f32, tag="acc")
                nc.vector.tensor_add(out=acc[:, :], in0=gts[0], in1=gts[1])
                nc.vector.tensor_add(out=acc[:, :], in0=acc[:, :], in1=gts[2])
                nc.vector.tensor_add(out=acc[:, :], in0=acc[:, :], in1=gts[3])
                nc.vector.tensor_add(out=acc[:, :], in0=acc[:, :], in1=gts[4])

                # scale by 1/n_ij and transpose (k h) -> h k in one DVE pass
                outb = bpool.tile([128, H, KC], f32, tag="outb")
                nc.vector.tensor_tensor(
                    out=outb[:, :, :],
                    in0=acc[:, :].rearrange("p (k h) -> p h k", h=H),
                    in1=rcp[:, :, c * KC:(c + 1) * KC],
                    op=mybir.AluOpType.mult,
                )

                # partition p = u*64 + i owns j = u*256 + c*128 + k
                for u in range(2):
                    j0 = u * KTOT + c * KC
                    nc.sync.dma_start(
                        out=out_t[:, :, j0:j0 + KC].rearrange("h i k -> i h k"),
                        in_=outb[u * RPC:(u + 1) * RPC, :, :],
                    )
    nc.compile()
    return nc


def _host_prep(edge_features_s, edge_weights, shortest_path_edges):
    feats = np.asarray(edge_features_s, dtype=np.float32)
    ew = np.asarray(edge_weights, dtype=np.float32)
    spe = np.asarray(shortest_path_edges).astype(np.int64)

    # table: row l*BLK + c = T_l[c-1] (c>=1) in cols 0:8, zeros at c=0 (padding)
    W = ew[1:L + 1].reshape(L, H, D)
    T = np.einsum("ed,lhd->leh", feats, W)            # [L, E, H]
    tblc = np.zeros((L * BLK, ROWW), np.float32)
    tblc.reshape(L, BLK, ROWW)[:, 1:, :H] = T

    comb = (spe + 1).astype(np.int16)                 # [N, N, L], 0 = padding
    recip = 1.0 / np.maximum((comb != 0).sum(-1), 1).astype(np.float32)  # [N, N]

    scol = NIDX // 16
    idx_all = np.empty((NCORES, 128, NCHUNK * L * scol), np.int16)
    rcp_all = np.empty((NCORES, 128, H, KTOT), np.float32)
    for cdev in range(NCORES):
        sub = comb[cdev * RPC:(cdev + 1) * RPC]       # [64, 512, L]
        # [i, u, c, kk, l] -> stream val[c, l, kk, u*64+i]
        a = sub.reshape(RPC, 2, NCHUNK, KC, L)
        val = np.transpose(a, (2, 4, 3, 1, 0)).reshape(NCHUNK, L, NIDX)
        # wrap: stream position s*16 + p16 sits at [p16, s]
        wrapped = val.reshape(NCHUNK, L, scol, 16).transpose(0, 1, 3, 2)
        idx_all[cdev] = np.tile(
            wrapped.transpose(2, 0, 1, 3).reshape(16, NCHUNK * L * scol),
            (8, 1),
        )
        r = recip[cdev * RPC:(cdev + 1) * RPC].reshape(RPC, 2, KTOT)
        rcp_all[cdev] = np.broadcast_to(
            np.transpose(r, (1, 0, 2)).reshape(128, 1, KTOT), (128, H, KTOT)
        )
    return tblc, idx_all, rcp_all


def kernel(edge_features_s, edge_weights, shortest_path_edges):
    if "nc" not in _cached:
        _cached["nc"] = build_nc()
    nc = _cached["nc"]

    tblc, idx_all, rcp_all = _host_prep(
        edge_features_s, edge_weights, shortest_path_edges
    )
    in_maps = []
    for c in range(NCORES):
        in_maps.append({
            "idxw": np.ascontiguousarray(idx_all[c]),
            "rcpt": np.ascontiguousarray(rcp_all[c]),
            "tblc": tblc,
        })
    res = run_bass_kernel_spmd(nc, in_maps, list(range(NCORES)))
    outs = [res.results[c]["out"].reshape(H, RPC, N) for c in range(NCORES)]
    return np.concatenate(outs, axis=1)


# revision 15
# speedup vs baseline: 1.2015x; 1.2015x over previous
"""EdgeEncoder kernel for Trainium2 (8 NeuronCores, row-sharded).

Reference (per pair (i, j) of an N x N grid):
    out[h, i, j] = (1/n_ij) * sum_l mask[i,j,l] * sum_d feats[idx[i,j,l], d] * W[l, h, d]
with n_ij = max(#valid l, 1), idx in [-1, E-1], -1 = padding.

Strategy: host projects the edge table into T_l[e, 0:8] rows stored in HBM at a
256 B row stride (row 0 of each l-block zeroed for padding).  Each core owns 64
i-rows and fetches its 64*512*5 = 163840 rows with SWDGE dma_gather
(InstDMAGatherAnt, non-transpose): descriptors are generated on Q7 at
~0.34 ns/desc and executed across all 16 DMA engines, so the random-access
gather runs at DMA descriptor rate instead of the Q7 ap_gather SBUF-port rate
(~21 ns/idx) of the previous version.  Rows carry only 32 B of payload
(elem_size=8 < the 256 B the bass wrapper insists on, but the ucode only
requires the row *stride* to be a 256 B multiple — a raw InstDMAGatherAnt
bypasses the wrapper assert).

Gather stream position i lands at dst[i%128, i//128, :]; the host orders each
(chunk, l) stream so partition p = u*64+i holds pairs j = u*256 + c*KCC + kk.
DVE reduces the 5 l-tiles, multiplies by host-computed 1/n_ij (shipped
pre-broadcast over h in [p, h, k] layout, so the same op also produces the
output-major order), and results DMA out as 512 B-contiguous j-runs.
"""

import numpy as np

import concourse.bass as bass
import concourse.mybir as mybir
import concourse.tile as tile
from concourse import bacc
from concourse.bass_utils import run_bass_kernel_spmd
from concourse._compat import exact_div

N, L, H, D, E = 512, 5, 8, 16, 10000
NCORES = 8
RPC = N // NCORES            # 64 rows (i) per core
BLK = E + 1                  # rows per l-block of the table
ROWW = 64                    # f32 per table row in HBM (256 B stride)
ELEM = 8                     # f32 actually fetched per row (32 B)
KTOT = N * RPC // 128        # 256 k-columns (pairs per partition)
# Q7 scratch holds num_idxs int32 + 512 B of swizzles in 65472 B, so keep
# num_idxs per gather at 8192 (32 KB staged indices).
NCHUNK = 4
KC = KTOT // NCHUNK          # 128 k-columns per chunk
NIDX = 128 * KC              # idxs per gather instruction
f32, i16 = mybir.dt.float32, mybir.dt.int16
# tile locks DMASW sems (8, rotating with instruction order) to one SWDGE
# queue each, so the queue pattern must be periodic with period dividing 8
QMAP = [[1, 2, 3, 0, 1, 2, 3, 1][m % 8] for m in range(20)]

_cached = {}


def _dma_gather_raw(nc, out_ap, in_ap, idxs_ap, num_idxs, elem_size, elem_step,
                    queue_num=0, single_packet=False):
    """nc.gpsimd.dma_gather(transpose=False) minus the elem_size%256 assert.

    The 256 B requirement is only on the HBM row *stride* (stride_bytes_256
    descriptor field); per-row payload is a plain SDMA descriptor length.
    """
    g = nc.gpsimd
    assert idxs_ap.dtype == i16
    assert in_ap.ap[0][0] == elem_step and in_ap.ap[-1][1] == elem_size
    stride_bytes_256 = exact_div(elem_step * mybir.dt.size(in_ap.dtype), 256)
    _in_ap = g.lower_ap_dma(in_ap, for_custom_bir_dma=True)
    _idxs_ap = g.lower_ap(idxs_ap)
    _out_ap = g.lower_ap(out_ap)
    return g.add_instruction(
        mybir.InstDMAGatherAnt(
            name=nc.get_next_instruction_name(),
            ins=[*_in_ap, _idxs_ap, g.lower_val_access(g.to_reg(num_idxs))],
            outs=[_out_ap],
            transpose=False,
            num_idxs=num_idxs,
            elem_size=elem_size,
            stride_bytes_256=stride_bytes_256,
            gen_mode=0,
            single_packet=single_packet,
            queue_num=queue_num,
            sbuf_tokens_per_rank=0,
            sbuf_free_dim_per_rank=0,
            sbuf_free_dim_pad_per_rank=0,
            sbuf_byte_offset=0,
        )
    )


def build_nc():
    nc = bacc.Bacc(dynamic_dma_scratch_size=1 << 15, num_swdge_queues=4)

    idx_t = nc.dram_tensor("idxw", [128, NCHUNK * L * (NIDX // 16)], i16,
                           kind="ExternalInput")
    rcp_t = nc.dram_tensor("rcpt", [128, H, KTOT], f32, kind="ExternalInput")
    tbl_t = nc.dram_tensor("tblc", [L * BLK, ROWW], f32, kind="ExternalInput")
    out_t = nc.dram_tensor("out", [H, RPC, N], f32, kind="ExternalOutput")

    with tile.TileContext(nc) as tc:
        with (
            tc.tile_pool(name="io", bufs=1) as iopool,
            tc.tile_pool(name="gth", bufs=2) as gpool,
            tc.tile_pool(name="acc", bufs=2) as apool,
            tc.tile_pool(name="outb", bufs=2) as bpool,
        ):
            idx = iopool.tile([128, NCHUNK * L * (NIDX // 16)], i16)
            half = NCHUNK * L * (NIDX // 16) // 2
            nc.sync.dma_start(out=idx[:, 0:half], in_=idx_t[:, 0:half])
            nc.sync.dma_start(out=idx[:, half:], in_=idx_t[:, half:])
            rcp = iopool.tile([128, H, KTOT], f32)
            nc.sync.dma_start(out=rcp[:, :, :], in_=rcp_t[:, :, :])

            scol = NIDX // 16
            for c in range(NCHUNK):
                gts = []
                for l in range(L):
                    gt = gpool.tile([128, KC, ELEM], f32, tag=f"g{l}")
                    s0 = (c * L + l) * scol
                    _dma_gather_raw(
                        nc,
                        out_ap=gt[:, :, :],
                        in_ap=tbl_t[l * BLK:(l + 1) * BLK, 0:ELEM],
                        idxs_ap=idx[:, s0:s0 + scol],
                        num_idxs=NIDX,
                        elem_size=ELEM,
                        elem_step=ROWW,
                        # queue 0 gathers hold the Pool engine for their full
                        # desc-gen (~64 us); queues 1-3 run mostly async on
                        # their Q7 core pairs but need q0's engine-holds as
                        # slack between their dispatches. 4 gathers on q0,
                        # the rest interleaved on 1-3.
                        queue_num=QMAP[c * L + l],
                    )
                    gts.append(gt[:, :, 0:H].rearrange("p k h -> p (k h)"))

                acc = apool.tile([128, KC * H], f32, tag="acc")
                nc.vector.tensor_add(out=acc[:, :], in0=gts[0], in1=gts[1])
                nc.vector.tensor_add(out=acc[:, :], in0=acc[:, :], in1=gts[2])
                nc.vector.tensor_add(out=acc[:, :], in0=acc[:, :], in1=gts[3])
                nc.vector.tensor_add(out=acc[:, :], in0=acc[:, :], in1=gts[4])

                # scale by 1/n_ij and transpose (k h) -> h k in one DVE pass
                outb = bpool.tile([128, H, KC], f32, tag="outb")
                nc.vector.tensor_tensor(
                    out=outb[:, :, :],
                    in0=acc[:, :].rearrange("p (k h) -> p h k", h=H),
                    in1=rcp[:, :, c * KC:(c + 1) * KC],
                    op=mybir.AluOpType.mult,
                )

                # partition p = u*64 + i owns j = u*256 + c*128 + k
                for u in range(2):
                    j0 = u * KTOT + c * KC
                    nc.sync.dma_start(
                        out=out_t[:, :, j0:j0 + KC].rearrange("h i k -> i h k"),
                        in_=outb[u * RPC:(u + 1) * RPC, :, :],
                    )
    nc.compile()
    return nc


def _host_prep(edge_features_s, edge_weights, shortest_path_edges):
    feats = np.asarray(edge_features_s, dtype=np.float32)
    ew = np.asarray(edge_weights, dtype=np.float32)
    spe = np.asarray(shortest_path_edges).astype(np.int64)

    # table: row l*BLK + c = T_l[c-1] (c>=1) in cols 0:8, zeros at c=0 (padding)
    W = ew[1:L + 1].reshape(L, H, D)
    T = np.einsum("ed,lhd->leh", feats, W)            # [L, E, H]
    tblc = np.zeros((L * BLK, ROWW), np.float32)
    tblc.reshape(L, BLK, ROWW)[:, 1:, :H] = T

    comb = (spe + 1).astype(np.int16)                 # [N, N, L], 0 = padding
    recip = 1.0 / np.maximum((comb != 0).sum(-1), 1).astype(np.float32)  # [N, N]

    scol = NIDX // 16
    idx_all = np.empty((NCORES, 128, NCHUNK * L * scol), np.int16)
    rcp_all = np.empty((NCORES, 128, H, KTOT), np.float32)
    for cdev in range(NCORES):
        sub = comb[cdev * RPC:(cdev + 1) * RPC]       # [64, 512, L]
        # [i, u, c, kk, l] -> stream val[c, l, kk, u*64+i]
        a = sub.reshape(RPC, 2, NCHUNK, KC, L)
        val = np.transpose(a, (2, 4, 3, 1, 0)).reshape(NCHUNK, L, NIDX)
        # wrap: stream position s*16 + p16 sits at [p16, s]
        wrapped = val.reshape(NCHUNK, L, scol, 16).transpose(0, 1, 3, 2)
        idx_all[cdev] = np.tile(
            wrapped.transpose(2, 0, 1, 3).reshape(16, NCHUNK * L * scol),
            (8, 1),
        )
        r = recip[cdev * RPC:(cdev + 1) * RPC].reshape(RPC, 2, KTOT)
        rcp_all[cdev] = np.broadcast_to(
            np.transpose(r, (1, 0, 2)).reshape(128, 1, KTOT), (128, H, KTOT)
        )
    return tblc, idx_all, rcp_all


def kernel(edge_features_s, edge_weights, shortest_path_edges):
    if "nc" not in _cached:
        _cached["nc"] = build_nc()
    nc = _cached["nc"]

    tblc, idx_all, rcp_all = _host_prep(
        edge_features_s, edge_weights, shortest_path_edges
    )
    in_maps = []
    for c in range(NCORES):
        in_maps.append({
            "idxw": np.ascontiguousarray(idx_all[c]),
            "rcpt": np.ascontiguousarray(rcp_all[c]),
            "tblc": tblc,
        })
    res = run_bass_kernel_spmd(nc, in_maps, list(range(NCORES)))
    outs = [res.results[c]["out"].reshape(H, RPC, N) for c in range(NCORES)]
    return np.concatenate(outs, axis=1)


# revision 21
# speedup vs baseline: 1.3020x; 1.0836x over previous
"""EdgeEncoder kernel for Trainium2 (8 NeuronCores, row-sharded).

Reference (per pair (i, j) of an N x N grid):
    out[h, i, j] = (1/n_ij) * sum_l mask[i,j,l] * sum_d feats[idx[i,j,l], d] * W[l, h, d]
with n_ij = max(#valid l, 1), idx in [-1, E-1], -1 = padding.

Strategy: host projects the edge table into T_l[e, 0:8] rows stored in HBM at a
256 B row stride (row 0 of each l-block zeroed for padding).  Each core owns 64
i-rows and fetches its 64*512*5 = 163840 rows with SWDGE dma_gather
(InstDMAGatherAnt, non-transpose): descriptors are generated on Q7 at
~0.34 ns/desc and executed across all 16 DMA engines, so the random-access
gather runs at DMA descriptor rate instead of the Q7 ap_gather SBUF-port rate
(~21 ns/idx) of the previous version.  Rows carry only 32 B of payload
(elem_size=8 < the 256 B the bass wrapper insists on, but the ucode only
requires the row *stride* to be a 256 B multiple — a raw InstDMAGatherAnt
bypasses the wrapper assert).

Gather stream position i lands at dst[i%128, i//128, :]; the host orders each
(chunk, l) stream so partition p = u*64+i holds pairs j = u*256 + c*KCC + kk.
DVE reduces the 5 l-tiles, multiplies by host-computed 1/n_ij (shipped
pre-broadcast over h in [p, h, k] layout, so the same op also produces the
output-major order), and results DMA out as 512 B-contiguous j-runs.
"""

import numpy as np

import concourse.bass as bass
import concourse.mybir as mybir
import concourse.tile as tile
from concourse import bacc
from concourse.bass_utils import run_bass_kernel_spmd
from concourse._compat import exact_div

N, L, H, D, E = 512, 5, 8, 16, 10000
NCORES = 8
RPC = N // NCORES            # 64 rows (i) per core
BLK = E + 1                  # rows per l-block of the table
ROWW = 64                    # f32 per table row in HBM (256 B stride)
ELEM = 8                     # f32 actually fetched per row (32 B)
KTOT = N * RPC // 128        # 256 k-columns (pairs per partition)
# Queue q runs on Q7 cores {2q, 2q+1}. q0 gathers hold the Pool engine for
# their full desc-gen (~7.8 ns/desc) while q1-3 dispatch quickly and
# generate concurrently (~3.3 ns/desc) on their core pairs — but their
# dispatches need q0 engine-holds as slack or they turn synchronous.
# So q0 gets the small chunk-0 gathers (2048 idxs) and queues 1-3 take one
# big chunk each (10240 idxs; Q7 scratch caps staged indices at ~16K).
# Emission interleaves small,q1,q2,q3,small,... — tile locks each of its 8
# rotating DMASW sems to one SWDGE queue, so the pattern must stay periodic.
NCHUNK = 4
KCS = [16, 80, 80, 80]                    # k-columns per chunk
KOFF = [0, 16, 96, 176]
KCMAX = max(KCS)
f32, i16 = mybir.dt.float32, mybir.dt.int16

_cached = {}


def _dma_gather_raw(nc, out_ap, in_ap, idxs_ap, num_idxs, elem_size, elem_step,
                    queue_num=0, single_packet=False):
    """nc.gpsimd.dma_gather(transpose=False) minus the elem_size%256 assert.

    The 256 B requirement is only on the HBM row *stride* (stride_bytes_256
    descriptor field); per-row payload is a plain SDMA descriptor length.
    """
    g = nc.gpsimd
    assert idxs_ap.dtype == i16
    assert in_ap.ap[0][0] == elem_step and in_ap.ap[-1][1] == elem_size
    stride_bytes_256 = exact_div(elem_step * mybir.dt.size(in_ap.dtype), 256)
    _in_ap = g.lower_ap_dma(in_ap, for_custom_bir_dma=True)
    _idxs_ap = g.lower_ap(idxs_ap)
    _out_ap = g.lower_ap(out_ap)
    return g.add_instruction(
        mybir.InstDMAGatherAnt(
            name=nc.get_next_instruction_name(),
            ins=[*_in_ap, _idxs_ap, g.lower_val_access(g.to_reg(num_idxs))],
            outs=[_out_ap],
            transpose=False,
            num_idxs=num_idxs,
            elem_size=elem_size,
            stride_bytes_256=stride_bytes_256,
            gen_mode=0,
            single_packet=single_packet,
            queue_num=queue_num,
            sbuf_tokens_per_rank=0,
            sbuf_free_dim_per_rank=0,
            sbuf_free_dim_pad_per_rank=0,
            sbuf_byte_offset=0,
        )
    )


def build_nc():
    nc = bacc.Bacc(dynamic_dma_scratch_size=1 << 15, num_swdge_queues=4)

    idx_t = nc.dram_tensor("idxw", [128, L * KTOT * 8], i16,
                           kind="ExternalInput")
    rcp_t = nc.dram_tensor("rcpt", [128, H, KTOT], f32, kind="ExternalInput")
    tbl_t = nc.dram_tensor("tblc", [L * BLK, ROWW], f32, kind="ExternalInput")
    out_t = nc.dram_tensor("out", [H, RPC, N], f32, kind="ExternalOutput")

    with tile.TileContext(nc) as tc:
        with (
            tc.tile_pool(name="io", bufs=1) as iopool,
            tc.tile_pool(name="gth", bufs=2) as gpool,
            tc.tile_pool(name="acc", bufs=2) as apool,
            tc.tile_pool(name="outb", bufs=2) as bpool,
        ):
            idx = iopool.tile([128, L * KTOT * 8], i16)
            nc.sync.dma_start(out=idx[:, :], in_=idx_t[:, :])
            rcp = iopool.tile([128, H, KTOT], f32)
            nc.sync.dma_start(out=rcp[:, :, :], in_=rcp_t[:, :, :])

            ioff = [sum(L * KCS[cc] * 8 for cc in range(c)) for c in range(NCHUNK)]
            order = []
            smalls = [(0, l) for l in range(L)]
            bigs = [(c, l) for c in range(1, NCHUNK) for l in range(L)]
            for slot in range(20):
                order.append(smalls.pop(0) if slot % 4 == 0 else bigs.pop(0))
            qmap = [slot % 4 for slot in range(20)]

            gts = {}
            done = {c: 0 for c in range(NCHUNK)}
            for slot, (c, l) in enumerate(order):
                kc = KCS[c]
                scol = kc * 8
                tag = f"s{l}" if c == 0 else f"g{l}"
                gt = gpool.tile([128, kc, ELEM], f32, tag=tag)
                s0 = ioff[c] + l * scol
                _dma_gather_raw(
                    nc,
                    out_ap=gt[:, :, :],
                    in_ap=tbl_t[l * BLK:(l + 1) * BLK, 0:ELEM],
                    idxs_ap=idx[:, s0:s0 + scol],
                    num_idxs=kc * 128,
                    elem_size=ELEM,
                    elem_step=ROWW,
                    queue_num=qmap[slot],
                )
                gts[(c, l)] = gt[:, :, 0:H].rearrange("p k h -> p (k h)")
                done[c] += 1
                if done[c] < L:
                    continue

                g = [gts[(c, ll)] for ll in range(L)]
                acc = apool.tile([128, kc * H], f32, tag="acc" if c else "acc0")
                nc.vector.tensor_add(out=acc[:, :], in0=g[0], in1=g[1])
                nc.vector.tensor_add(out=acc[:, :], in0=acc[:, :], in1=g[2])
                nc.vector.tensor_add(out=acc[:, :], in0=acc[:, :], in1=g[3])
                nc.vector.tensor_add(out=acc[:, :], in0=acc[:, :], in1=g[4])

                # scale by 1/n_ij and transpose (k h) -> h k in one DVE pass
                outb = bpool.tile([128, H, kc], f32, tag="outb" if c else "outb0")
                nc.vector.tensor_tensor(
                    out=outb[:, :, :],
                    in0=acc[:, :].rearrange("p (k h) -> p h k", h=H),
                    in1=rcp[:, :, KOFF[c]:KOFF[c] + kc],
                    op=mybir.AluOpType.mult,
                )

                # partition p = u*64 + i owns j = u*256 + KOFF[c] + k
                for u in range(2):
                    j0 = u * KTOT + KOFF[c]
                    nc.sync.dma_start(
                        out=out_t[:, :, j0:j0 + kc].rearrange("h i k -> i h k"),
                        in_=outb[u * RPC:(u + 1) * RPC, :, :],
                    )
    nc.compile()
    return nc


def _host_prep(edge_features_s, edge_weights, shortest_path_edges):
    feats = np.asarray(edge_features_s, dtype=np.float32)
    ew = np.asarray(edge_weights, dtype=np.float32)
    spe = np.asarray(shortest_path_edges).astype(np.int64)

    # table: row l*BLK + c = T_l[c-1] (c>=1) in cols 0:8, zeros at c=0 (padding)
    W = ew[1:L + 1].reshape(L, H, D)
    T = np.einsum("ed,lhd->leh", feats, W)            # [L, E, H]
    tblc = np.zeros((L * BLK, ROWW), np.float32)
    tblc.reshape(L, BLK, ROWW)[:, 1:, :H] = T

    comb = (spe + 1).astype(np.int16)                 # [N, N, L], 0 = padding
    recip = 1.0 / np.maximum((comb != 0).sum(-1), 1).astype(np.float32)  # [N, N]

    idx_all = np.empty((NCORES, 128, L * KTOT * 8), np.int16)
    rcp_all = np.empty((NCORES, 128, H, KTOT), np.float32)
    for cdev in range(NCORES):
        sub = comb[cdev * RPC:(cdev + 1) * RPC]       # [64, 512, L]
        sj = sub.reshape(RPC, 2, KTOT, L)             # [i, u, kglobal, l]
        parts = []
        for c in range(NCHUNK):
            v = sj[:, :, KOFF[c]:KOFF[c] + KCS[c], :]  # [i, u, kk, l]
            # stream position kk*128 + u*64 + i
            val = np.transpose(v, (3, 2, 1, 0)).reshape(L, KCS[c] * 128)
            scol = KCS[c] * 8
            # wrap: stream position s*16 + p16 sits at [p16, s]
            wrapped = val.reshape(L, scol, 16).transpose(0, 2, 1)
            parts.append(wrapped.transpose(1, 0, 2).reshape(16, L * scol))
        idx_all[cdev] = np.tile(np.concatenate(parts, axis=1), (8, 1))
        r = recip[cdev * RPC:(cdev + 1) * RPC].reshape(RPC, 2, KTOT)
        rcp_all[cdev] = np.broadcast_to(
            np.transpose(r, (1, 0, 2)).reshape(128, 1, KTOT), (128, H, KTOT)
        )
    return tblc, idx_all, rcp_all


def kernel(edge_features_s, edge_weights, shortest_path_edges):
    if "nc" not in _cached:
        _cached["nc"] = build_nc()
    nc = _cached["nc"]

    tblc, idx_all, rcp_all = _host_prep(
        edge_features_s, edge_weights, shortest_path_edges
    )
    in_maps = []
    for c in range(NCORES):
        in_maps.append({
            "idxw": np.ascontiguousarray(idx_all[c]),
            "rcpt": np.ascontiguousarray(rcp_all[c]),
            "tblc": tblc,
        })
    res = run_bass_kernel_spmd(nc, in_maps, list(range(NCORES)))
    outs = [res.results[c]["out"].reshape(H, RPC, N) for c in range(NCORES)]
    return np.concatenate(outs, axis=1)


# revision 31
# speedup vs baseline: 1.3761x; 1.0569x over previous
"""EdgeEncoder kernel for Trainium2 (8 NeuronCores, row-sharded).

Reference (per pair (i, j) of an N x N grid):
    out[h, i, j] = (1/n_ij) * sum_l mask[i,j,l] * sum_d feats[idx[i,j,l], d] * W[l, h, d]
with n_ij = max(#valid l, 1), idx in [-1, E-1], -1 = padding.

Strategy: host projects the edge table into T_l[e, 0:8] rows stored in HBM at a
256 B row stride (row 0 of each l-block zeroed for padding).  Each core owns 64
i-rows and fetches its 64*512*5 = 163840 rows with SWDGE dma_gather
(InstDMAGatherAnt, non-transpose): descriptors are generated on Q7 at
~0.34 ns/desc and executed across all 16 DMA engines, so the random-access
gather runs at DMA descriptor rate instead of the Q7 ap_gather SBUF-port rate
(~21 ns/idx) of the previous version.  Rows carry only 32 B of payload
(elem_size=8 < the 256 B the bass wrapper insists on, but the ucode only
requires the row *stride* to be a 256 B multiple — a raw InstDMAGatherAnt
bypasses the wrapper assert).

Gather stream position i lands at dst[i%128, i//128, :]; the host orders each
(chunk, l) stream so partition p = u*64+i holds pairs j = u*256 + c*KCC + kk.
DVE reduces the 5 l-tiles, multiplies by host-computed 1/n_ij (shipped
pre-broadcast over h in [p, h, k] layout, so the same op also produces the
output-major order), and results DMA out as 512 B-contiguous j-runs.
"""

import numpy as np

import concourse.bass as bass
import concourse.mybir as mybir
import concourse.tile as tile
from concourse import bacc
from concourse.bass_utils import run_bass_kernel_spmd
from concourse._compat import exact_div

N, L, H, D, E = 512, 5, 8, 16, 10000
NCORES = 8
RPC = N // NCORES            # 64 rows (i) per core
BLK = E + 1                  # rows per l-block of the table
ROWW = 64                    # f32 per table row in HBM (256 B stride)
ELEM = 8                     # f32 actually fetched per row (32 B)
KTOT = N * RPC // 128        # 256 k-columns (pairs per partition)
# Queue q runs on Q7 cores {2q, 2q+1}. q0 gathers hold the Pool engine for
# their full desc-gen (~7.8 ns/desc) while q1-3 dispatch in a few us and
# generate async (~3.3 ns/desc) on their core pairs — but a dispatch to a
# still-busy pair blocks the Pool NX head-of-line, so q0's engine-holds must
# provide >= one async gen time of slack per dispatch round. Chunk 0 (small,
# q0's five gathers, ~32 us hold each) paces rounds of three big-chunk
# gathers on q1-3 (~31 us gen each). Q7 scratch caps staged idxs at ~16K.
NCHUNK = 4
KCS = [32, 75, 75, 74]       # k-columns per chunk
KOFF = [0, 32, 107, 182]
KCMAX = max(KCS[1:])
f32, i16 = mybir.dt.float32, mybir.dt.int16

_cached = {}


def _dma_gather_raw(nc, out_ap, in_ap, idxs_ap, num_idxs, elem_size, elem_step,
                    queue_num=0, single_packet=False):
    """nc.gpsimd.dma_gather(transpose=False) minus the elem_size%256 assert.

    The 256 B requirement is only on the HBM row *stride* (stride_bytes_256
    descriptor field); per-row payload is a plain SDMA descriptor length.
    """
    g = nc.gpsimd
    assert idxs_ap.dtype == i16
    assert in_ap.ap[0][0] == elem_step and in_ap.ap[-1][1] == elem_size
    stride_bytes_256 = exact_div(elem_step * mybir.dt.size(in_ap.dtype), 256)
    _in_ap = g.lower_ap_dma(in_ap, for_custom_bir_dma=True)
    _idxs_ap = g.lower_ap(idxs_ap)
    _out_ap = g.lower_ap(out_ap)
    return g.add_instruction(
        mybir.InstDMAGatherAnt(
            name=nc.get_next_instruction_name(),
            ins=[*_in_ap, _idxs_ap, g.lower_val_access(g.to_reg(num_idxs))],
            outs=[_out_ap],
            transpose=False,
            num_idxs=num_idxs,
            elem_size=elem_size,
            stride_bytes_256=stride_bytes_256,
            gen_mode=0,
            single_packet=single_packet,
            queue_num=queue_num,
            sbuf_tokens_per_rank=0,
            sbuf_free_dim_per_rank=0,
            sbuf_free_dim_pad_per_rank=0,
            sbuf_byte_offset=0,
        )
    )


def build_nc():
    nc = bacc.Bacc(dynamic_dma_scratch_size=1 << 15, num_swdge_queues=4)

    idx_t = nc.dram_tensor("idxw", [128, L * KTOT * 8], i16,
                           kind="ExternalInput")
    rcp_t = nc.dram_tensor("rcpt", [128, H, KTOT], f32, kind="ExternalInput")
    tbl_t = nc.dram_tensor("tblc", [L * BLK, ROWW], f32, kind="ExternalInput")
    out_t = nc.dram_tensor("out", [H, RPC, N], f32, kind="ExternalOutput")

    with tile.TileContext(nc) as tc:
        with (
            tc.tile_pool(name="io", bufs=1) as iopool,
            tc.tile_pool(name="gth", bufs=2) as gpool,
            tc.tile_pool(name="acc", bufs=2) as apool,
            tc.tile_pool(name="outb", bufs=2) as bpool,
        ):
            idx = iopool.tile([128, L * KTOT * 8], i16)
            nc.sync.dma_start(out=idx[:, :], in_=idx_t[:, :])
            rcp = iopool.tile([128, H, KTOT], f32)
            nc.sync.dma_start(out=rcp[:, :, :], in_=rcp_t[:, :, :])

            ioff = [sum(L * KCS[cc] * 8 for cc in range(c))
                    for c in range(NCHUNK)]
            smalls = [(0, l) for l in range(L)]
            bigs = [(c, l) for c in range(1, NCHUNK) for l in range(L)]
            order = [(smalls if s % 4 == 0 else bigs).pop(0)
                     for s in range(20)]

            gts = {}
            done = {c: 0 for c in range(NCHUNK)}
            for slot, (c, l) in enumerate(order):
                kc = KCS[c]
                scol = kc * 8
                if c == 0:
                    gt = gpool.tile([128, KCS[0], ELEM], f32, tag=f"s{l}")
                else:
                    gt = gpool.tile([128, KCMAX, ELEM], f32, tag=f"g{l}")
                s0 = ioff[c] + l * scol
                _dma_gather_raw(
                    nc,
                    out_ap=gt[:, 0:kc, :],
                    in_ap=tbl_t[l * BLK:(l + 1) * BLK, 0:ELEM],
                    idxs_ap=idx[:, s0:s0 + scol],
                    num_idxs=kc * 128,
                    elem_size=ELEM,
                    elem_step=ROWW,
                    queue_num=slot % 4,
                )
                gts[(c, l)] = gt[:, 0:kc, 0:H].rearrange("p k h -> p (k h)")
                done[c] += 1
                if done[c] < L:
                    continue

                g = [gts[(c, ll)] for ll in range(L)]
                acc = apool.tile([128, KCMAX * H], f32,
                                 tag="acc" if c else "acc0")
                a = acc[:, 0:kc * H]
                nc.vector.tensor_add(out=a, in0=g[0], in1=g[1])
                nc.vector.tensor_add(out=a, in0=a, in1=g[2])
                nc.vector.tensor_add(out=a, in0=a, in1=g[3])
                nc.vector.tensor_add(out=a, in0=a, in1=g[4])

                # scale by 1/n_ij and transpose (k h) -> h k in one DVE pass
                outb = bpool.tile([128, H, KCMAX], f32,
                                  tag="outb" if c else "outb0")
                nc.vector.tensor_tensor(
                    out=outb[:, :, 0:kc],
                    in0=a.rearrange("p (k h) -> p h k", h=H),
                    in1=rcp[:, :, KOFF[c]:KOFF[c] + kc],
                    op=mybir.AluOpType.mult,
                )

                # partition p = u*64 + i owns j = u*256 + KOFF[c] + k
                for u in range(2):
                    j0 = u * KTOT + KOFF[c]
                    nc.sync.dma_start(
                        out=out_t[:, :, j0:j0 + kc].rearrange("h i k -> i h k"),
                        in_=outb[u * RPC:(u + 1) * RPC, :, 0:kc],
                    )
    nc.compile()
    return nc


def _host_prep(edge_features_s, edge_weights, shortest_path_edges):
    feats = np.asarray(edge_features_s, dtype=np.float32)
    ew = np.asarray(edge_weights, dtype=np.float32)
    spe = np.asarray(shortest_path_edges).astype(np.int64)

    # table: row l*BLK + c = T_l[c-1] (c>=1) in cols 0:8, zeros at c=0 (padding)
    W = ew[1:L + 1].reshape(L, H, D)
    T = np.einsum("ed,lhd->leh", feats, W)            # [L, E, H]
    tblc = np.zeros((L * BLK, ROWW), np.float32)
    tblc.reshape(L, BLK, ROWW)[:, 1:, :H] = T

    comb = (spe + 1).astype(np.int16)                 # [N, N, L], 0 = padding
    recip = 1.0 / np.maximum((comb != 0).sum(-1), 1).astype(np.float32)  # [N, N]

    idx_all = np.empty((NCORES, 128, L * KTOT * 8), np.int16)
    rcp_all = np.empty((NCORES, 128, H, KTOT), np.float32)
    for cdev in range(NCORES):
        sub = comb[cdev * RPC:(cdev + 1) * RPC]       # [64, 512, L]
        sj = sub.reshape(RPC, 2, KTOT, L)             # [i, u, kglobal, l]
        parts = []
        for c in range(NCHUNK):
            v = sj[:, :, KOFF[c]:KOFF[c] + KCS[c], :]  # [i, u, kk, l]
            # stream position kk*128 + u*64 + i
            val = np.transpose(v, (3, 2, 1, 0)).reshape(L, KCS[c] * 128)
            scol = KCS[c] * 8
            # wrap: stream position s*16 + p16 sits at [p16, s]
            wrapped = val.reshape(L, scol, 16).transpose(0, 2, 1)
            parts.append(wrapped.transpose(1, 0, 2).reshape(16, L * scol))
        idx_all[cdev] = np.tile(np.concatenate(parts, axis=1), (8, 1))
        r = recip[cdev * RPC:(cdev + 1) * RPC].reshape(RPC, 2, KTOT)
        rcp_all[cdev] = np.broadcast_to(
            np.transpose(r, (1, 0, 2)).reshape(128, 1, KTOT), (128, H, KTOT)
        )
    return tblc, idx_all, rcp_all


def kernel(edge_features_s, edge_weights, shortest_path_edges):
    if "nc" not in _cached:
        _cached["nc"] = build_nc()
    nc = _cached["nc"]

    tblc, idx_all, rcp_all = _host_prep(
        edge_features_s, edge_weights, shortest_path_edges
    )
    in_maps = []
    for c in range(NCORES):
        in_maps.append({
            "idxw": np.ascontiguousarray(idx_all[c]),
            "rcpt": np.ascontiguousarray(rcp_all[c]),
            "tblc": tblc,
        })
    res = run_bass_kernel_spmd(nc, in_maps, list(range(NCORES)))
    outs = [res.results[c]["out"].reshape(H, RPC, N) for c in range(NCORES)]
    return np.concatenate(outs, axis=1)
